# revision 7
# baseline (speedup 1.0000x reference)
"""Chamfer-loss min/argmin kernel for Trainium2 (8 NeuronCores).

Problem: preds [4, 8192, 3], gts [4, 8192, 3] fp32.
P[b, n, m] = ||gts[b,n]||^2 + ||preds[b,m]||^2 - 2 <gts[b,n], preds[b,m]>
Outputs: (min over n [4,8192], min over m [4,8192],
          argmin over n int32, argmin over m int32).

Sharding: 8 cores = 4 batches x 2 halves of the gts (n) axis. Each core
holds full preds for its batch and a 4096-row slice of gts. Per-gt-row
results (min over m) are final; per-pred-row results (min over n) are
partial over the n-slice and combined on the host.

Device kernel per core (both directions, roles swapped):
 - K=5 fp32r matmuls (rows [-2x0,-2x1,-2x2, 1, rx] x [y0,y1,y2, ry, 1])
   produce the FULL distance matrix P directly in PSUM (both norms fold
   into the contraction; fp32r runs the PE at 4x the fp32 rate and its
   ~2^-13 relative error vanishes under the 2^-12 pack quantization).
   Matmuls are quad-packed with tile_position row groups filling one
   [128, 2048] PSUM group per quad.
 - The scalar (ACT) and gpsimd (Pool) engines stage the ODD columns of
   each PSUM group to SBUF (a DVE instruction may read only one PSUM
   operand).
 - ONE custom DVE op (CHAMFER_PAIR_PACK_MIN) per group then consumes
   column PAIRS — Src0 = even columns straight from PSUM (stride-2),
   Src1 = staged odd columns — packing the column index into the low
   11 mantissa bits and min-reducing, 2 elements/cycle:
     pe = (P_even & ~0x7FF) | 2k     [k from an exact denormal scan]
     po = (P_odd  & ~0x7FF) | 2k+1
     out = min(pe, po); accum = min(out)  -> [128, 1] per group
   fp32 ordering of packed values equals ordering of quantized P (all
   P > 0), ties resolve to the smallest column (first occurrence).
   The host decodes value/argmin from the accum bits; no max_index
   pass, no full-row staging.
"""

import functools

import numpy as np

BS, N, M, D = 4, 8192, 8192, 3
NSL = N // 2  # gts rows per core
K = 5  # contraction: 3 coords + both norms
N_CORES = 8
GROUP = 2048  # PSUM group: 4 banks of 512 fp32
MASK11 = np.uint32(0x7FF)


def _register_op():
    """Register the CHAMFER_PAIR_PACK_MIN custom DVE op (idempotent).

    C0 = 0x7FF mask bits (denormal), C1 = -2 ulp (denormal scan step),
    imm2 = accum seed. The scan generates bits 0x7FF - 2k by exact
    denormal integer arithmetic; XOR against (P | 0x7FF) clears the low
    11 bits of P and installs the column index in one step.
    """
    import concourse.dve_ops as dvo
    from concourse.dve_spec import (Spec, Src0, Src1, C0, C1, C2, Bin, AluOp,
                                    minn, lower, Scan, _has_src1, Leaf)
    from concourse.dve_uop import DveOpSpec, InpSel
    from concourse.dve_table_gen import dve_ver_for

    name = "CHAMFER_PAIR_PACK_MIN"
    for op in dvo.OPS:
        if op.name == name:
            return op

    init = Bin(AluOp.SUBTRACT, C0, C1)     # 0x7FF - (-2) = 0x801
    xi_e = Scan(AluOp.ADD, C1, init=init)  # bits = 0x7FF - 2k
    xi_o = Bin(AluOp.BITWISE_XOR, xi_e, Leaf(InpSel.ONE_U32))
    q0 = Bin(AluOp.BITWISE_OR, Src0, C0)
    p0 = Bin(AluOp.BITWISE_XOR, q0, xi_e)
    q1 = Bin(AluOp.BITWISE_OR, Src1, C0)
    p1 = Bin(AluOp.BITWISE_XOR, q1, xi_o)
    body = Bin(AluOp.MIN, p0, p1)

    def ref(in0, in1, c0, c1, c2):
        a = np.ascontiguousarray(in0, np.float32).view(np.uint32)
        b = np.ascontiguousarray(in1, np.float32).view(np.uint32)
        m = MASK11
        L = a.shape[-1]
        k2 = (2 * np.arange(L, dtype=np.uint32))[None, :]
        pe = ((a & ~m) | k2).view(np.float32)
        po = ((b & ~m) | (k2 + 1)).view(np.float32)
        out = np.minimum(pe, po)
        P = out.shape[0]
        acc = np.minimum(out.reshape(P, -1).min(-1, keepdims=True),
                         np.float32(c2))
        return out, acc

    spec = Spec(body=body, accum=minn, accum_init=C2, reference=ref)
    ver = dve_ver_for("TRN2")
    row = dvo._CUSTOM_DVE_ROW_BASE + len(dvo.OPS)
    uops = lower(spec, ver=ver)
    ds = DveOpSpec(name=name, opcode=row, uops=uops, rd1_en=_has_src1(spec))
    op = dvo.DveOp(name, spec, subdim=False, uops_sha={ver: ds.sha(ver)})
    dvo.OPS.append(op)
    dvo._SUB_OPCODE_FOR_NAME[op.name] = row
    dvo.CUSTOM_DVE_SPECS[op.name] = spec
    return op


def _register_single_op():
    """Single-stream variant: packed = (P & ~0x7FF) | k, accum = min.
    One element/cycle straight from PSUM, no staging needed. Used on a
    fraction of groups to offload the scalar engine. C1 = -1 ulp.
    """
    import concourse.dve_ops as dvo
    from concourse.dve_spec import (Spec, Src0, C0, C1, C2, Bin, AluOp,
                                    minn, lower, Scan, _has_src1)
    from concourse.dve_uop import DveOpSpec
    from concourse.dve_table_gen import dve_ver_for

    name = "CHAMFER_SINGLE_PACK_MIN"
    for op in dvo.OPS:
        if op.name == name:
            return op

    init = Bin(AluOp.SUBTRACT, C0, C1)     # 0x7FF + 1 = 0x800
    xi = Scan(AluOp.ADD, C1, init=init)    # bits = 0x7FF - k
    q = Bin(AluOp.BITWISE_OR, Src0, C0)
    body = Bin(AluOp.BITWISE_XOR, q, xi)

    def ref(in0, in1, c0, c1, c2):
        a = np.ascontiguousarray(in0, np.float32).view(np.uint32)
        m = MASK11
        L = a.shape[-1]
        k = np.arange(L, dtype=np.uint32)[None, :]
        out = ((a & ~m) | k).view(np.float32)
        P = out.shape[0]
        acc = np.minimum(out.reshape(P, -1).min(-1, keepdims=True),
                         np.float32(c2))
        return out, acc

    spec = Spec(body=body, accum=minn, accum_init=C2, reference=ref)
    ver = dve_ver_for("TRN2")
    row = dvo._CUSTOM_DVE_ROW_BASE + len(dvo.OPS)
    uops = lower(spec, ver=ver)
    ds = DveOpSpec(name=name, opcode=row, uops=uops, rd1_en=_has_src1(spec))
    op = dvo.DveOp(name, spec, subdim=False, uops_sha={ver: ds.sha(ver)})
    dvo.OPS.append(op)
    dvo._SUB_OPCODE_FOR_NAME[op.name] = row
    dvo.CUSTOM_DVE_SPECS[op.name] = spec
    return op


# odd slots are ACT-staged and pair-packed; even slots single-pack from PSUM


def _build_nc(nsl, m, reps=1):
    import contextlib

    import concourse.bacc as bacc
    import concourse.mybir as mybir
    import concourse.tile as tile

    f32 = mybir.dt.float32
    f32r = mybir.dt.float32r
    pack_op = _register_op()
    single_op = _register_single_op()

    nc = bacc.Bacc("TRN2", target_bir_lowering=False, debug=False)

    ga = nc.declare_dram_parameter("ga", [2 * K, nsl], f32, isOutput=False)
    pa = nc.declare_dram_parameter("pa", [2 * K, m], f32, isOutput=False)
    m07 = nc.declare_dram_parameter("m07", [128, 1], f32, isOutput=False)
    mstep = nc.declare_dram_parameter("mstep", [128, 1], f32, isOutput=False)
    mstep1 = nc.declare_dram_parameter("mstep1", [128, 1], f32, isOutput=False)
    n_gacc = (nsl // 128) * (m // GROUP)
    n_pacc = (m // 128) * (nsl // GROUP)
    gacc_o = nc.declare_dram_parameter("gacc", [128, n_gacc], f32, isOutput=True)
    pacc_o = nc.declare_dram_parameter("pacc", [128, n_pacc], f32, isOutput=True)

    with tile.TileContext(nc) as tc:
        with (
            tc.tile_pool(name="const", bufs=1) as const,
            tc.tile_pool(name="scr", bufs=3) as scr,
            tc.tile_pool(name="oddp", bufs=3) as oddp,
            tc.tile_pool(name="outs", bufs=1) as outs,
            tc.tile_pool(name="psum", bufs=2, space="PSUM") as psum,
        ):
            # operands replicated into the 4 PE row groups (partitions 32j,
            # 32-aligned as the matmul requires), one tensor per operand role
            ga_repL = const.tile([128, nsl], f32)
            ga_repR = const.tile([128, nsl], f32)
            pa_repR = const.tile([128, m], f32)
            pa_repL = const.tile([128, m], f32)
            for j in range(4):
                nc.sync.dma_start(ga_repL[32 * j : 32 * j + K, :], ga[0:K, :])
                nc.sync.dma_start(ga_repR[32 * j : 32 * j + K, :], ga[K : 2 * K, :])
                nc.sync.dma_start(pa_repR[32 * j : 32 * j + K, :], pa[0:K, :])
                nc.sync.dma_start(pa_repL[32 * j : 32 * j + K, :], pa[K : 2 * K, :])
            m07_sb = const.tile([128, 1], f32)
            step_sb = const.tile([128, 1], f32)
            step1_sb = const.tile([128, 1], f32)
            nc.sync.dma_start(m07_sb[:], m07[:])
            nc.sync.dma_start(step_sb[:], mstep[:])
            nc.sync.dma_start(step1_sb[:], mstep1[:])

            rep_loop = tc.For_i(0, reps, 1) if reps > 1 else contextlib.nullcontext()
            rep_loop.__enter__()

            gacc_sb = outs.tile([128, n_gacc], f32)
            pacc_sb = outs.tile([128, n_pacc], f32)
            stage_ctr = [0]

            def direction(n_chunks, lhs_rep, rhs_rep, rhs_len, acc_sb):
                n_groups = rhs_len // GROUP
                for ci in range(n_chunks):
                    for g in range(n_groups):
                        pt = psum.tile([128, GROUP], f32, tag="pt")
                        for j in range(4):
                            lhsT = lhs_rep[32 * j : 32 * j + K,
                                           ci * 128 : (ci + 1) * 128]
                            col0 = g * GROUP + j * 512
                            nc.tensor.matmul(
                                pt[:, j * 512 : (j + 1) * 512],
                                lhsT=lhsT,
                                rhs=rhs_rep[32 * j : 32 * j + K,
                                            col0 : col0 + 512],
                                start=True,
                                stop=True,
                                tile_position=(32 * j, 0),
                            )
                        s = ci * n_groups + g
                        if s % 2 == 0:
                            # single-stream: 1 elem/cyc from PSUM, no staging
                            st2 = scr.tile([128, GROUP], f32, tag="st2")
                            nc.vector._custom_dve(
                                single_op, out=st2[:], in0=pt[:],
                                s0=m07_sb[:, 0:1], s1=step1_sb[:, 0:1],
                                imm2=3.0e38,
                                accum_out=acc_sb[:, s : s + 1],
                            )
                        else:
                            # ACT copies the whole group to SBUF (freeing the
                            # PSUM buffer early so the next matmuls overlap
                            # the DVE), then DVE pair-packs the SBUF halves
                            # (k, 1024+k) at 2 elems/cycle
                            stg = oddp.tile([128, GROUP], f32, tag="stg")
                            nc.scalar.activation(
                                stg[:], pt[:],
                                mybir.ActivationFunctionType.Identity,
                            )
                            st = scr.tile([128, GROUP // 2], f32, tag="st")
                            nc.vector._custom_dve(
                                pack_op, out=st[:], in0=stg[:, 0 : GROUP // 2],
                                in1=stg[:, GROUP // 2 : GROUP],
                                s0=m07_sb[:, 0:1], s1=step_sb[:, 0:1],
                                imm2=3.0e38,
                                accum_out=acc_sb[:, s : s + 1],
                            )

            # per-gt rows: min/argmin over preds (final)
            direction(nsl // 128, ga_repL, pa_repR, m, gacc_sb)
            # per-pred rows: min/argmin over the gts slice (partial)
            direction(m // 128, pa_repL, ga_repR, nsl, pacc_sb)

            nc.sync.dma_start(gacc_o[:], gacc_sb[:])
            nc.sync.dma_start(pacc_o[:], pacc_sb[:])

            rep_loop.__exit__(None, None, None)
    nc.finalize()
    return nc


@functools.lru_cache(maxsize=None)
def _get_nc(nsl, m, reps=1):
    return _build_nc(nsl, m, reps)


def _augment(preds_b, gts_bh):
    """Operands for the K=5 scheme (norms folded into the matmul).

    ga rows 0-4: [-2x0, -2x1, -2x2, 1, rx]  (lhsT, per-gt direction)
    ga rows 5-9: [-2x0, -2x1, -2x2, rx, 1]  (rhs,  per-pred direction)
    pa rows 0-4: [y0, y1, y2, ry, 1]        (rhs,  per-gt direction)
    pa rows 5-9: [y0, y1, y2, 1, ry]        (lhsT, per-pred direction)
    matmul: P[n, m] = -2<x_n, y_m> + rx[n] + ry[m] directly in PSUM.
    """
    x = np.ascontiguousarray(gts_bh, dtype=np.float32)
    y = np.ascontiguousarray(preds_b, dtype=np.float32)
    nsl = x.shape[0]
    m = y.shape[0]
    rx = (x[:, 0] * x[:, 0] + x[:, 1] * x[:, 1] + x[:, 2] * x[:, 2]).astype(np.float32)
    ry = (y[:, 0] * y[:, 0] + y[:, 1] * y[:, 1] + y[:, 2] * y[:, 2]).astype(np.float32)
    ga = np.empty((2 * K, nsl), np.float32)
    ga[0:3] = (np.float32(-2.0) * x).T
    ga[3] = 1.0
    ga[4] = rx
    ga[5:8] = ga[0:3]
    ga[8] = rx
    ga[9] = 1.0
    pa = np.empty((2 * K, m), np.float32)
    pa[0:3] = y.T
    pa[3] = ry
    pa[4] = 1.0
    pa[5:8] = y.T
    pa[8] = 1.0
    pa[9] = ry
    return ga, pa


_M07 = np.full((128, 1), MASK11, dtype=np.uint32).view(np.float32)
_MSTEP = np.full((128, 1), np.uint32(0x80000002), dtype=np.uint32).view(np.float32)
_MSTEP1 = np.full((128, 1), np.uint32(0x80000001), dtype=np.uint32).view(np.float32)


@functools.lru_cache(maxsize=None)
def _get_dispatcher(nsl, m, reps=1):
    """Build the SPMD PJRT dispatcher once and cache it (the stock
    run_bass_via_pjrt re-traces jax.jit on every call)."""
    import jax
    import numpy as _np
    from jax.sharding import Mesh, PartitionSpec
    from jax.experimental.shard_map import shard_map
    import concourse.mybir as mybir
    from concourse import bass2jax

    bass2jax.install_neuronx_cc_hook()
    nc = _get_nc(nsl, m, reps)

    partition_name = nc.partition_id_tensor.name if nc.partition_id_tensor else None
    in_names, out_names, out_avals, zero_outs = [], [], [], []
    for alloc in nc.m.functions[0].allocations:
        if not isinstance(alloc, mybir.MemoryLocationSet):
            continue
        name = alloc.memorylocations[0].name
        if alloc.kind == "ExternalInput":
            if name != partition_name:
                in_names.append(name)
        elif alloc.kind == "ExternalOutput":
            shape = tuple(alloc.tensor_shape)
            dtype = mybir.dt.np(alloc.dtype)
            out_names.append(name)
            out_avals.append(jax.core.ShapedArray(shape, dtype))
            zero_outs.append(_np.zeros(shape, dtype))
    n_params = len(in_names)
    n_outs = len(out_avals)
    all_in_names = list(in_names) + list(out_names)
    if partition_name is not None:
        all_in_names.append(partition_name)
    donate = tuple(range(n_params, n_params + n_outs))

    def _body(*args):
        operands = list(args)
        if partition_name is not None:
            operands.append(bass2jax.partition_id_tensor())
        outs = bass2jax._bass_exec_p.bind(
            *operands,
            out_avals=tuple(out_avals),
            in_names=tuple(all_in_names),
            out_names=tuple(out_names),
            lowering_input_output_aliases=(),
            sim_require_finite=True,
            sim_require_nnan=True,
            nc=nc,
        )
        return tuple(outs)

    devices = jax.devices()[:N_CORES]
    mesh = Mesh(np.asarray(devices), ("core",))
    in_specs = (PartitionSpec("core"),) * (n_params + n_outs)
    out_specs = (PartitionSpec("core"),) * n_outs
    sharded = jax.jit(
        shard_map(_body, mesh=mesh, in_specs=in_specs, out_specs=out_specs,
                  check_rep=False),
        donate_argnums=donate,
        keep_unused=True,
    )

    def dispatch(in_maps):
        concat_in = [
            np.concatenate([np.asarray(in_maps[c][nm]) for c in range(N_CORES)], axis=0)
            for nm in in_names
        ]
        concat_zeros = [
            np.zeros((N_CORES * z.shape[0], *z.shape[1:]), z.dtype) for z in zero_outs
        ]
        out_arrs = sharded(*concat_in, *concat_zeros)
        return [
            {nm: np.asarray(out_arrs[i]).reshape(N_CORES, *out_avals[i].shape)[c]
             for i, nm in enumerate(out_names)}
            for c in range(N_CORES)
        ]

    return dispatch


def _make_in_maps(preds, gts):
    in_maps = []
    for c in range(N_CORES):
        b, h = c // 2, c % 2
        ga, pa = _augment(preds[b], gts[b, h * NSL : (h + 1) * NSL])
        in_maps.append({"ga": ga, "pa": pa, "m07": _M07, "mstep": _MSTEP,
                        "mstep1": _MSTEP1})
    return in_maps


def _decode(acc, n_rows, n_groups):
    """acc [128, (n_rows//128)*n_groups] packed fp32 -> (val, col) per row.

    Row layout: partition p, slot ci*n_groups+g  ->  row ci*128 + p.
    Low 11 bits of each accum hold 2k (first-half column k) or 2k+1
    (second-half column 1024+k); group slots cover disjoint ascending
    column blocks, so argmin over the quantized values (first
    occurrence) gives the smallest global column block among ties.
    """
    n_chunks = n_rows // 128
    a = acc.reshape(128, n_chunks, n_groups).transpose(1, 0, 2).reshape(n_rows, n_groups)
    bits = np.ascontiguousarray(a).view(np.uint32)
    vq = (bits & ~MASK11).view(np.float32)
    j = vq.argmin(axis=1)
    rows = np.arange(n_rows)
    b11 = bits[rows, j] & MASK11
    slot = (rows // 128) * n_groups + j
    is_pair = (slot % 2) == 1
    col_in_group = np.where(is_pair, (b11 >> 1) + (b11 & 1) * (GROUP // 2), b11)
    col = col_in_group.astype(np.int32) + (j * GROUP).astype(np.int32)
    return vq[rows, j], col


def kernel(preds, gts, mask):
    preds = np.asarray(preds, dtype=np.float32)
    gts = np.asarray(gts, dtype=np.float32)

    results = _get_dispatcher(NSL, M)(_make_in_maps(preds, gts))

    out_pmin = np.empty((BS, M), np.float32)
    out_gmin = np.empty((BS, N), np.float32)
    out_pidx = np.empty((BS, M), np.int32)
    out_gidx = np.empty((BS, N), np.int32)

    for b in range(BS):
        r0, r1 = results[2 * b], results[2 * b + 1]
        # per-gt rows (min over preds): each half is final
        for h, r in ((0, r0), (1, r1)):
            gm, gi = _decode(r["gacc"], NSL, M // GROUP)
            out_gmin[b, h * NSL : (h + 1) * NSL] = gm
            out_gidx[b, h * NSL : (h + 1) * NSL] = gi
        # per-pred rows: combine the two n-halves
        pm0, pi0 = _decode(r0["pacc"], M, NSL // GROUP)
        pm1, pi1 = _decode(r1["pacc"], M, NSL // GROUP)
        take1 = pm1 < pm0  # tie -> half 0 (lower gt index), first occurrence
        out_pmin[b] = np.where(take1, pm1, pm0)
        out_pidx[b] = np.where(take1, pi1 + NSL, pi0)

    return out_pmin, out_gmin, out_pidx, out_gidx
